# revision 1
# baseline (speedup 1.0000x reference)
"""Trainium2 Bass kernel for windowed (banded) self-attention MLP block.

Reference computation (per batch b):
    h = relu(x @ W1 + b1)                      # [S, H]
    q = h @ Wq                                 # [S, H]
    scores[s, w] = q[s] . h_pad[s + w] / 32    # window w in [0, 33), h zero-padded by A=16
    wgt = softmax(scores, axis=w)
    out[s] = sum_w wgt[s, w] * h_pad[s + w]

Sharding: 8 cores, each takes 1024 consecutive tokens of the flattened
[B*S] = 8192 token stream (2 cores per batch element; shards never cross a
batch boundary).  Each core redundantly computes h for a 16-token halo on
each side, so no cross-core communication is needed.

Per-core DRAM layouts (host prepares; 'aug' = augmented with a validity
row of 1.0s in x and the b1 row in W1 so h = relu(x_aug @ W1_aug)):
    xa  [128, 5, 1152] bf16   x_aug^T chunked along IN (4 chunks + aug chunk)
    w1  [8, 128, 5, 128] bf16 W1_aug chunked hc-major (for early-start DMA)
    wq  [128, 8, 1024] bf16   (Wq / 32) chunked along H_in
    b1c [128, 8] f32          b1 as per-hc bias columns
    hm  [128, 2] f32          halo validity multipliers (left, right)
    out [1024, 1024] bf16     (host casts back to f32)

On-chip stages (bf16 matmul operands, fp32 PSUM accumulation):
    A:  hT[hc, t] = relu(W1^T @ xT + b1)   H-on-partitions, 1056 tokens,
        bias+relu fused in one DVE tensor_scalar; halo cols zeroed via hm
    B:  qT[ho, t] = (Wq/32)^T @ hT         core 1024 tokens
    C:  h[t, hc]  = relu(xT_aug^T @ W1_aug)  token-on-partitions recompute
        (aug form: out-of-range tokens come out exactly 0)
    D:  per 128-token tile: scores = qT^T @ hT_window  [128, 160]
        p = exp(scores + bandmask) (bf16) + denominator via ACT accum_out,
        pT via PE transpose; out = (pT^T @ h_window) * (1/den)
"""

import sys

import numpy as np

try:
    import concourse.bass as bass
except ImportError:
    sys.path.insert(0, "/opt/trn_rl_repo")
    import concourse.bass as bass

import ml_dtypes

import concourse.mybir as mybir
import concourse.tile as tile
from concourse import bacc
from concourse.bass_utils import run_bass_kernel_spmd

BF16 = ml_dtypes.bfloat16

B, S, IN, H = 4, 2048, 512, 1024
A = 16
WND = 2 * A + 1            # 33 window positions
NCORES = 8
TOK = (B * S) // NCORES    # 1024 tokens per core
TOKH = TOK + 2 * A         # 1056 with halo
TOKP = 9 * 128             # 1152 zero-padded token slots
NT = TOK // 128            # 8 output tiles per core
WIN = 128 + 2 * A          # 160-token window per 128-token tile
NEG = -30000.0             # additive mask for out-of-band positions

f32 = mybir.dt.float32
bf16 = mybir.dt.bfloat16
AF = mybir.ActivationFunctionType
ALU = mybir.AluOpType


def _band_mask():
    """[128, WIN] additive mask: row t allows window cols t..t+32."""
    m = np.full((128, WIN), NEG, dtype=np.float32)
    for t in range(128):
        m[t, t : t + WND] = 0.0
    return m


def _kernel_body(tc, nc, xa_d, w1_d, wq_d, b1c_d, b1b_d, hm_d, out_d, mask_d, id_d):
    with (
        tc.tile_pool(name="const", bufs=1) as cpool,
        tc.tile_pool(name="wts", bufs=1) as wpool,
        tc.tile_pool(name="acts", bufs=1) as apool,
        tc.tile_pool(name="dc", bufs=2) as dcpool,
    ):
        # Load order matters: stage A's first matmul group needs only
        # xa piece 0 and w1 chunk 0 — issue those first, bulk later.
        xa = wpool.tile([128, 5, TOKP], bf16, tag="xa")
        w1 = wpool.tile([128, 8, 5, 128], bf16, tag="w1")
        wq = wpool.tile([128, 8, H], bf16, tag="wq")
        b1c = cpool.tile([128, 8], f32, tag="b1c")
        hm = cpool.tile([128, 2], f32, tag="hm")
        # DMA issue costs ~0.6us each on the Sync queue, so order by when the
        # data is first needed: stage A tile 0 is only 128 tokens wide and
        # needs just xa[:, 0:4, 0:128] and w1[hc=0, c<4]; aug rows (c=4, for
        # stage C) and the bulk come later.
        # scratch tile for PE warm-up matmuls (HAM clock-gate release)
        warm = wpool.tile([128, 512], bf16, tag="warm")
        nc.vector.memset(warm[:], 0.0)

        nc.sync.dma_start(xa[:, 0:4, 0:128], xa_d[:, 0:4, 0:128])
        nc.sync.dma_start(w1[:, 0, 0:4], w1_d[0, :, 0:4])
        nc.sync.dma_start(b1c[:], b1c_d[:])
        nc.sync.dma_start(xa[:, 0:4, 128:640], xa_d[:, 0:4, 128:640])
        for hc in range(1, 4):
            nc.sync.dma_start(w1[:, hc, 0:4], w1_d[hc, :, 0:4])
        for hc in range(4, 8):
            nc.sync.dma_start(w1[:, hc, 0:4], w1_d[hc, :, 0:4])
        nc.sync.dma_start(xa[:, 0:4, 640:TOKP], xa_d[:, 0:4, 640:TOKP])
        nc.sync.dma_start(xa[:, 4, :], xa_d[:, 4, :])
        b1b = cpool.tile([128, 8, 128], bf16, tag="b1b")
        nc.sync.dma_start(b1b[:], b1b_d[:])
        nc.sync.dma_start(w1[:, :, 4], w1_d[:, :, 4].rearrange("h p j -> p h j"))
        nc.sync.dma_start(hm[:], hm_d[:])
        mask_sb = cpool.tile([128, WIN], f32, tag="mask")
        nc.sync.dma_start(mask_sb[:], mask_d[:])
        id_sb = cpool.tile([128, 128], bf16, tag="ident")
        nc.sync.dma_start(id_sb[:], id_d[:])
        nc.sync.dma_start(wq[:], wq_d[:])

        hT = apool.tile([128, 8, TOKH], bf16, tag="hT")
        hh = apool.tile([128, 9, H], bf16, tag="hh")
        qT = apool.tile([128, 8, TOK], bf16, tag="qT")

        # ---- stages A, B, C (big dense matmuls) ----
        with tc.tile_pool(name="psABC", bufs=1, space="PSUM") as psABC:
            # PE warm-up: ~12 matmuls on a zeroed scratch tile keep the PE
            # busy during the initial input-DMA wait, so the HAM clock gate
            # is already released (2.4 GHz) when the real matmuls start.
            for _ in range(12):
                wps = psABC.tile([128, 512], f32, tag="warm", bufs=1)
                nc.tensor.matmul(
                    wps[:], warm[:, 0:128], warm[:], start=True, stop=True
                )
            # A: hT = relu(W1^T @ xT + b1); token tiles (128, 512, 416) —
            # a small first tile so the first matmul group's DMA wait is short.
            # bias+relu fused on DVE: (psum + b1) max 0 -> bf16
            A_TILES = ((0, 128), (128, 640), (640, TOKH))
            for t0, t1 in A_TILES:
                sl = slice(t0, t1)
                for hc in range(8):
                    ps = psABC.tile([128, t1 - t0], f32, tag="pa", bufs=2)
                    for c in range(4):
                        nc.tensor.matmul(
                            ps[:],
                            w1[:, hc, c, :],
                            xa[:, c, sl],
                            start=(c == 0),
                            stop=(c == 3),
                        )
                    nc.vector.tensor_scalar(
                        hT[:, hc, sl],
                        ps[:],
                        b1c[:, hc : hc + 1],
                        0.0,
                        ALU.add,
                        ALU.max,
                    )
            # zero the halo columns that fall outside this core's batch
            for hc in range(8):
                nc.vector.tensor_scalar_mul(
                    hT[:, hc, 0:A], hT[:, hc, 0:A], hm[:, 0:1]
                )
                nc.vector.tensor_scalar_mul(
                    hT[:, hc, TOK + A : TOKH], hT[:, hc, TOK + A : TOKH],
                    hm[:, 1:2],
                )

            # C: h = relu(x^T_aug @ W1_aug), 9 token tiles of 128.
            # Boundary tiles (0, 8) use the aug chunk (handles bias AND
            # out-of-range zeros).  Interior tiles are always fully valid,
            # so they skip the aug matmul (saves 512 PE cycles/group) and
            # apply bias+relu on DVE instead (b1 row broadcast from w1).
            for t in range(9):
                tsl = slice(t * 128, (t + 1) * 128)
                boundary = t in (0, 8)
                nch = 5 if boundary else 4
                for half in range(2):
                    hsl = slice(half * 512, (half + 1) * 512)
                    ps = psABC.tile([128, 512], f32, tag="pc", bufs=2)
                    for c in range(nch):
                        nc.tensor.matmul(
                            ps[:],
                            xa[:, c, tsl],
                            w1[:, half * 4 : (half + 1) * 4, c, :],
                            start=(c == 0),
                            stop=(c == nch - 1),
                        )
                    if boundary:
                        nc.scalar.activation(hh[:, t, hsl], ps[:], AF.Relu)
                    else:
                        zz = dcpool.tile([128, 512], f32, tag="zz")
                        nc.vector.tensor_add(
                            zz[:], ps[:], b1b[:, half * 4 : (half + 1) * 4, :]
                        )
                        nc.vector.tensor_scalar_max(hh[:, t, hsl], zz[:], 0.0)

            # B: qT = (Wq/32)^T @ hT for the core 1024 tokens
            for th in range(2):
                off = th * 512
                for ho in range(8):
                    ps = psABC.tile([128, 512], f32, tag="pb", bufs=2)
                    for hi in range(8):
                        nc.tensor.matmul(
                            ps[:],
                            wq[:, hi, ho * 128 : (ho + 1) * 128],
                            hT[:, hi, A + off : A + off + 512],
                            start=(hi == 0),
                            stop=(hi == 7),
                        )
                    nc.scalar.activation(qT[:, ho, off : off + 512], ps[:], AF.Copy)

        # ---- stage D: windowed attention per 128-token tile ----
        with (
            tc.tile_pool(name="psD", bufs=1, space="PSUM") as psD,
            tc.tile_pool(name="dtmp", bufs=2) as dpool,
            tc.tile_pool(name="outp", bufs=3) as opool,
        ):
            for T in range(NT):
                ps_s = psD.tile([128, WIN], f32, tag="ps", bufs=2)
                for hc in range(8):
                    nc.tensor.matmul(
                        ps_s[:],
                        qT[:, hc, T * 128 : (T + 1) * 128],
                        hT[:, hc, T * 128 : T * 128 + WIN],
                        start=(hc == 0),
                        stop=(hc == 7),
                    )
                s_sb = dpool.tile([128, WIN], f32, tag="s")
                nc.vector.tensor_add(s_sb[:], ps_s[:], mask_sb[:])
                p_sb = dpool.tile([128, WIN], bf16, tag="p")
                den = dpool.tile([128, 1], f32, tag="den")
                nc.scalar.activation(p_sb[:], s_sb[:], AF.Exp, accum_out=den[:])
                rcp = dpool.tile([128, 1], f32, tag="rcp")
                nc.vector.reciprocal(rcp[:], den[:])

                ptm = psD.tile([128, 256], bf16, tag="pt", bufs=2)
                nc.tensor.transpose(ptm[:, 0:128], p_sb[:, 0:128], id_sb[:])
                nc.tensor.transpose(ptm[0:32, 128:256], p_sb[:, 128:WIN], id_sb[:])
                pta_sb = dpool.tile([128, 256], bf16, tag="pta")
                nc.vector.tensor_copy(pta_sb[:, 0:128], ptm[:, 0:128])
                nc.vector.tensor_copy(pta_sb[0:32, 128:256], ptm[0:32, 128:256])

                out_sb = opool.tile([128, H], bf16, tag="osb")
                for half in range(2):
                    hsl = slice(half * 512, (half + 1) * 512)
                    pav = psD.tile([128, 512], f32, tag="pav", bufs=3)
                    nc.tensor.matmul(
                        pav[:], pta_sb[:, 0:128], hh[:, T, hsl],
                        start=True, stop=False,
                    )
                    nc.tensor.matmul(
                        pav[:], pta_sb[0:32, 128:256], hh[0:32, T + 1, hsl],
                        start=False, stop=True,
                    )
                    # alternate the normalize+copyback between DVE and ACT so
                    # the two halves run on different engines
                    if half == 0:
                        nc.vector.tensor_scalar_mul(out_sb[:, hsl], pav[:], rcp[:])
                    else:
                        nc.scalar.mul(out_sb[:, hsl], pav[:], rcp[:])
                    nc.sync.dma_start(
                        out_d[T * 128 : (T + 1) * 128, hsl], out_sb[:, hsl]
                    )


def build_nc():
    nc = bacc.Bacc("TRN2", target_bir_lowering=False, debug=False, num_devices=NCORES)
    xa_d = nc.dram_tensor("xa", [128, 5, TOKP], bf16, kind="ExternalInput")
    w1_d = nc.dram_tensor("w1", [8, 128, 5, 128], bf16, kind="ExternalInput")
    wq_d = nc.dram_tensor("wq", [128, 8, H], bf16, kind="ExternalInput")
    b1c_d = nc.dram_tensor("b1c", [128, 8], f32, kind="ExternalInput")
    b1b_d = nc.dram_tensor("b1b", [128, 8, 128], bf16, kind="ExternalInput")
    hm_d = nc.dram_tensor("hm", [128, 2], f32, kind="ExternalInput")
    out_d = nc.dram_tensor("out", [TOK, H], bf16, kind="ExternalOutput")
    mask_d = nc.inline_tensor(_band_mask(), "bandmask")
    id_d = nc.inline_tensor(np.eye(128, dtype=BF16), "ident")

    with tile.TileContext(nc) as tc:
        _kernel_body(tc, nc, xa_d, w1_d, wq_d, b1c_d, b1b_d, hm_d, out_d, mask_d, id_d)
    nc.compile()
    return nc


def make_inputs(x, W1, b1, Wq):
    """Host-side shard prep (numpy only; not part of HW time)."""
    x = np.asarray(x, dtype=np.float32)
    W1 = np.asarray(W1, dtype=np.float32)
    b1 = np.asarray(b1, dtype=np.float32)
    Wq = np.asarray(Wq, dtype=np.float32)

    # w1a[hc, p, c, j] = W1_aug[c*128 + p, hc*128 + j]
    w1a = np.zeros((8, 128, 5, 128), dtype=BF16)
    for hc in range(8):
        for c in range(4):
            w1a[hc, :, c, :] = W1[
                c * 128 : (c + 1) * 128, hc * 128 : (hc + 1) * 128
            ].astype(BF16)
        w1a[hc, 0, 4, :] = b1[hc * 128 : (hc + 1) * 128].astype(BF16)

    b1c = np.ascontiguousarray(b1.reshape(8, 128).T).astype(np.float32)  # [128, 8]
    b1b = np.ascontiguousarray(
        np.broadcast_to(b1.astype(BF16).reshape(1, 8, 128), (128, 8, 128))
    )

    wqs = (Wq / np.sqrt(np.float32(H))).astype(BF16)
    wqa = np.zeros((128, 8, H), dtype=BF16)
    for c in range(8):
        wqa[:, c, :] = wqs[c * 128 : (c + 1) * 128, :]

    in_maps = []
    for core in range(NCORES):
        b, half = divmod(core, 2)
        lo = half * TOK - A
        hi = half * TOK + TOK + A
        s0, s1 = max(lo, 0), min(hi, S)
        xs = np.zeros((TOKH, IN), dtype=np.float32)
        xs[s0 - lo : s1 - lo] = x[b, s0:s1]
        xT = np.ascontiguousarray(xs.T).astype(BF16)  # [512, 1056]
        xa = np.zeros((128, 5, TOKP), dtype=BF16)
        for c in range(4):
            xa[:, c, :TOKH] = xT[c * 128 : (c + 1) * 128, :]
        xa[0, 4, s0 - lo : s1 - lo] = BF16(1.0)
        hmv = np.zeros((128, 2), dtype=np.float32)
        hmv[:, 0] = 1.0 if lo >= 0 else 0.0
        hmv[:, 1] = 1.0 if hi <= S else 0.0
        in_maps.append({"xa": xa, "w1": w1a, "wq": wqa, "b1c": b1c, "b1b": b1b, "hm": hmv})
    return in_maps


_NC_CACHE = {}


def get_nc():
    if "nc" not in _NC_CACHE:
        _NC_CACHE["nc"] = build_nc()
    return _NC_CACHE["nc"]


def kernel(x, W1, b1, Wq, atten_size, _trace=False, _trace_kwargs=None):
    assert int(atten_size) == A, f"kernel hardcodes atten_size=16, got {atten_size}"
    nc = get_nc()
    in_maps = make_inputs(x, W1, b1, Wq)
    kw = {}
    if _trace:
        kw = dict(trace=True, trace_kwargs=_trace_kwargs or {})
    res = run_bass_kernel_spmd(nc, in_maps, core_ids=list(range(NCORES)), **kw)
    out = np.stack([r["out"].astype(np.float32) for r in res.results])
    out = out.reshape(B, S, H)
    if _trace:
        return out, res
    return out


if __name__ == "__main__":
    import jax

    key = jax.random.key(0)
    k1, k2, k3, k4 = jax.random.split(key, 4)
    x = np.asarray(jax.random.normal(k1, (B, S, IN), dtype=np.float32))
    W1 = np.asarray(
        jax.random.normal(k2, (IN, H), dtype=np.float32) * (1.0 / np.sqrt(IN))
    )
    b1 = np.asarray(jax.random.normal(k3, (H,), dtype=np.float32) * 0.02)
    Wq = np.asarray(
        jax.random.normal(k4, (H, H), dtype=np.float32) * (1.0 / np.sqrt(H))
    )
    out = kernel(x, W1, b1, Wq, 16)
    print("out", out.shape, out.dtype, float(np.abs(out).max()))

